# revision 1
# baseline (speedup 1.0000x reference)
"""GCN layer (x @ W -> edge gather/scale/scatter-add -> +bias, relu) on 8 NeuronCores.

Measured: ~650 us/iteration on 8 axon-tunneled trn2 cores, rel err 4.5e-07
vs the fp32 jax reference (timing via on-device repeat slope; the axon RPC
adds ~95 ms/call that the slope cancels).

Strategy (per sharding hint):
  - Shard nodes across 8 cores (6250 rows each). Each core computes its local
    xw = x_shard @ W on the PE (host pre-transposes x so K lands on
    partitions), then an AllGather builds the full xw table [50000, 64] in
    every core's DRAM.
  - Edges are partitioned by destination shard on the host, sorted by
    destination window (128 nodes), grouped into 128-edge groups that each
    target a single window.  Per group the core:
      * dma_gather's the 128 source rows (256B each) from the xw table
        (SWDGE custom gather, 4 SWDGE queues round-robin -- queue depth is
        the gather throughput lever: 33 GB/s at q=1 vs 323 GB/s at q=4)
      * builds a one-hot*val matrix [128e, 128d] with one fused DVE
        tensor_scalar (iota == dstloc) * val   (ohk=0 path; ohk>0 selects a
        bulk tensor_tensor builder, measured slower in situ)
      * accumulates psum[64f, 128d] += msgs[128e, 64f].T @ onehot on the PE
        (messages are the 64-col stationary operand, onehot streams)
    Window accumulation ends with a fused bias+relu on the scalar engine
    (bias is per-partition in the transposed layout) into an output staging
    tile; one DMA stores outT [64, 6250] and the host transposes.
  - int16 gather indices only address <32768 rows, so edges are split into a
    "low" stream (src < 25000) and "high" stream (src >= 25000), gathered
    from the matching half of the xw table.  Group counts per (window, half)
    are padded to the max over cores (~+13% edges) so all 8 cores run one
    SPMD program (run_bass_kernel_spmd shares a single instruction stream).
  - Host reassembles: out = concat(outT_c.T).
"""

import os
import sys

import numpy as np


def _ensure_concourse():
    try:
        import concourse  # noqa: F401
        return
    except ImportError:
        pass
    for p in ("/opt/trn_rl_repo", "/root/.axon_site/_ro/trn_rl_repo"):
        if os.path.isdir(p):
            sys.path.insert(0, p)
            try:
                import concourse  # noqa: F401
                return
            except ImportError:
                sys.path.pop(0)
    raise ImportError("concourse (bass) not found")


_ensure_concourse()

import concourse.bacc as bacc  # noqa: E402
import concourse.mybir as mybir  # noqa: E402
import concourse.tile as tile  # noqa: E402
from concourse import bass_utils  # noqa: E402

F32 = mybir.dt.float32
I16 = mybir.dt.int16
I32 = mybir.dt.int32


def _cdiv(a, b):
    return -(-a // b)


def preprocess(edge_src, edge_dst, edge_vals, *, n_nodes, cores, win):
    """Partition/sort/pad edges into per-core low/high streams.

    Returns a dict with SPMD-uniform structure (G arrays) and per-core data
    arrays laid out exactly as the device consumes them.
    """
    shard = n_nodes // cores
    nwin = _cdiv(shard, win)
    half = n_nodes // 2

    src = np.asarray(edge_src).astype(np.int64)
    dst = np.asarray(edge_dst).astype(np.int64)
    vals = np.asarray(edge_vals).astype(np.float32)
    e = src.shape[0]

    core = dst // shard
    dl = dst - core * shard
    w = dl // win
    h = (src >= half).astype(np.int64)
    key = (core * nwin + w) * 2 + h

    order = np.argsort(key, kind="stable")
    ks = key[order]
    src_s = src[order]
    dloc_s = (dl - w * win)[order].astype(np.float32)
    v_s = vals[order]
    c_s = core[order]
    w_s = w[order]
    h_s = h[order]

    nbuck = cores * nwin * 2
    sizes = np.bincount(key, minlength=nbuck)
    starts = np.concatenate(([0], np.cumsum(sizes)))[:-1]
    rank = np.arange(e, dtype=np.int64) - starts[ks]

    # groups per (window, half): max over cores
    cnt = sizes.reshape(cores, nwin, 2)
    G = _cdiv(cnt, 128).max(axis=0)  # [nwin, 2]
    glo, ghi = G[:, 0], G[:, 1]
    cum_lo = np.concatenate(([0], np.cumsum(glo)))  # group offsets per window
    cum_hi = np.concatenate(([0], np.cumsum(ghi)))
    gtot_lo, gtot_hi = int(cum_lo[-1]), int(cum_hi[-1])
    nlo, nhi = gtot_lo * 128, gtot_hi * 128

    idx_lo = np.zeros((cores, max(nlo, 1)), np.int16)
    dst_lo = np.zeros((cores, max(nlo, 1)), np.float32)
    val_lo = np.zeros((cores, max(nlo, 1)), np.float32)
    idx_hi = np.zeros((cores, max(nhi, 1)), np.int16)
    dst_hi = np.zeros((cores, max(nhi, 1)), np.float32)
    val_hi = np.zeros((cores, max(nhi, 1)), np.float32)

    m = h_s == 0
    pos = cum_lo[w_s[m]] * 128 + rank[m]
    idx_lo[c_s[m], pos] = src_s[m].astype(np.int16)
    dst_lo[c_s[m], pos] = dloc_s[m]
    val_lo[c_s[m], pos] = v_s[m]

    m = h_s == 1
    pos = cum_hi[w_s[m]] * 128 + rank[m]
    idx_hi[c_s[m], pos] = (src_s[m] - half).astype(np.int16)
    dst_hi[c_s[m], pos] = dloc_s[m]
    val_hi[c_s[m], pos] = v_s[m]

    def idx_layout(a, n):
        # logical position i -> [i % 16, i // 16], replicated to 128 partitions
        if n == 0:
            return None
        blk = a[:n].reshape(-1, 16).T  # [16, n/16]
        return np.ascontiguousarray(np.tile(blk, (8, 1)))  # [128, n/16]

    def grp_layout(a, n):
        # position g*128+p -> [p, g]
        if n == 0:
            return None
        return np.ascontiguousarray(a[:n].reshape(-1, 128).T)  # [128, G]

    return dict(
        shard=shard,
        nwin=nwin,
        half=half,
        glo=glo,
        ghi=ghi,
        cum_lo=cum_lo,
        cum_hi=cum_hi,
        gtot_lo=gtot_lo,
        gtot_hi=gtot_hi,
        nlo=nlo,
        nhi=nhi,
        idx_lo=[idx_layout(idx_lo[c], nlo) for c in range(cores)],
        dst_lo=[grp_layout(dst_lo[c], nlo) for c in range(cores)],
        val_lo=[grp_layout(val_lo[c], nlo) for c in range(cores)],
        idx_hi=[idx_layout(idx_hi[c], nhi) for c in range(cores)],
        dst_hi=[grp_layout(dst_hi[c], nhi) for c in range(cores)],
        val_hi=[grp_layout(val_hi[c], nhi) for c in range(cores)],
    )


def build_program(meta, *, n_nodes, din, dout, cores, win, maxb=64,
                  msgs_bufs=2, gp_build=0, sc_ps_bufs=4, debug_mode=None,
                  ohk=16, oh_bufs=3, act_build=0,
                  debug_skip_gemm=False, debug_skip_scatter=False, repeat=1):
    """Build the SPMD Bass program. Returns (nc, input_names)."""
    shard = meta["shard"]
    nwin = meta["nwin"]
    half = meta["half"]
    glo, ghi = meta["glo"], meta["ghi"]
    cum_lo, cum_hi = meta["cum_lo"], meta["cum_hi"]
    gtot_lo, gtot_hi = meta["gtot_lo"], meta["gtot_hi"]
    nlo, nhi = meta["nlo"], meta["nhi"]
    kch = _cdiv(din, 128)
    nr = _cdiv(shard, 128)

    nc = bacc.Bacc("TRN2", target_bir_lowering=False, debug=False,
                   num_devices=cores, num_swdge_queues=4)

    t_xT = nc.dram_tensor("xT", [din, shard], F32, kind="ExternalInput")
    t_w = nc.dram_tensor("w", [din, dout], F32, kind="ExternalInput")
    t_bias = nc.dram_tensor("bias", [dout, 1], F32, kind="ExternalInput")
    t_idx = {}
    t_dst = {}
    t_val = {}
    if nlo:
        t_idx["lo"] = nc.dram_tensor("idx_lo", [128, nlo // 16], I16, kind="ExternalInput")
        t_dst["lo"] = nc.dram_tensor("dst_lo", [128, gtot_lo], F32, kind="ExternalInput")
        t_val["lo"] = nc.dram_tensor("val_lo", [128, gtot_lo], F32, kind="ExternalInput")
    if nhi:
        t_idx["hi"] = nc.dram_tensor("idx_hi", [128, nhi // 16], I16, kind="ExternalInput")
        t_dst["hi"] = nc.dram_tensor("dst_hi", [128, gtot_hi], F32, kind="ExternalInput")
        t_val["hi"] = nc.dram_tensor("val_hi", [128, gtot_hi], F32, kind="ExternalInput")
    t_out = nc.dram_tensor("outT", [dout, shard], F32, kind="ExternalOutput")

    if debug_skip_gemm:
        t_xw_full = nc.dram_tensor("xw_full", [n_nodes, dout], F32,
                                   kind="ExternalInput")
    else:
        t_xw_shard = nc.dram_tensor("xw_shard", [shard, dout], F32)
        t_xw_full = nc.dram_tensor("xw_full", [n_nodes, dout], F32,
                                   addr_space="Shared" if cores > 4 else "Local")

    with tile.TileContext(nc) as tc:
        with (
            tc.tile_pool(name="const", bufs=1) as constp,
            tc.tile_pool(name="xt", bufs=1) as xtp,
            tc.tile_pool(name="stage", bufs=1) as stagep,
            tc.tile_pool(name="meta", bufs=1) as metap,
            tc.tile_pool(name="msgs_lo", bufs=msgs_bufs) as mlp,
            tc.tile_pool(name="msgs_hi", bufs=msgs_bufs) as mhp,
            tc.tile_pool(name="oh", bufs=oh_bufs) as ohp,
            tc.tile_pool(name="gemm_ps", bufs=2, space="PSUM") as gpsp,
            tc.tile_pool(name="sc_ps", bufs=sc_ps_bufs, space="PSUM") as spsp,
        ):
            # ---- constants ----
            # iota repeated max(ohk,1) times: iota_rep[p, k*win + j] = j
            iw = max(ohk, 1)
            iota_i = constp.tile([128, iw * win], I32)
            nc.gpsimd.iota(iota_i[:], pattern=[[0, iw], [1, win]], base=0,
                           channel_multiplier=0)
            iota_f = constp.tile([128, iw * win], F32)
            nc.vector.tensor_copy(iota_f[:], iota_i[:])
            bias_sb = constp.tile([dout, 1], F32)
            nc.sync.dma_start(bias_sb[:], t_bias[:])
            w_sb = constp.tile([128, kch * dout], F32)
            nc.sync.dma_start(
                w_sb[:].rearrange("p (k e) -> p k e", e=dout),
                t_w[:].rearrange("(k p) e -> p k e", p=128),
            )

            # ---- metadata loads ----
            sb_idx = {}
            sb_dst = {}
            sb_val = {}
            for s in t_idx:
                sb_idx[s] = metap.tile(list(t_idx[s].shape), I16, tag=f"idx_{s}", name=f"sb_idx_{s}")
                nc.sync.dma_start(sb_idx[s][:], t_idx[s][:])
                sb_dst[s] = metap.tile(list(t_dst[s].shape), F32, tag=f"dst_{s}", name=f"sb_dst_{s}")
                nc.sync.dma_start(sb_dst[s][:], t_dst[s][:])
                sb_val[s] = metap.tile(list(t_val[s].shape), F32, tag=f"val_{s}", name=f"sb_val_{s}")
                nc.sync.dma_start(sb_val[s][:], t_val[s][:])
            sb_negd = {}
            sb_negv = {}
            if act_build:
                for s in t_idx:
                    sb_negd[s] = metap.tile(list(t_dst[s].shape), F32,
                                            tag=f"negd_{s}", name=f"sb_negd_{s}")
                    nc.vector.tensor_scalar(
                        sb_negd[s][:], sb_dst[s][:], -1.0, None,
                        mybir.AluOpType.mult)
                    sb_negv[s] = metap.tile(list(t_val[s].shape), F32,
                                            tag=f"negv_{s}", name=f"sb_negv_{s}")
                    nc.vector.tensor_scalar(
                        sb_negv[s][:], sb_val[s][:], -1.0, None,
                        mybir.AluOpType.mult)

            # ---- per-iteration body (repeat>1 used only for timing) ----
            def emit_body(rep):
                # ---- local GEMM: xw_shard = x_shard @ W ----
                if debug_skip_gemm:
                    xt_sb = None
                else:
                    xt_sb = []
                    for k in range(kch):
                        kp = min(128, din - k * 128)
                        xt = xtp.tile([kp, shard], F32, tag=f"xt{k}")
                        nc.sync.dma_start(xt[:], t_xT[k * 128:k * 128 + kp, :])
                        xt_sb.append(xt)
                    xw_stage = stagep.tile([128, nr * dout], F32, tag="xw_stage")
                    for r in range(nr):
                        rw = min(128, shard - r * 128)
                        ps = gpsp.tile([rw, dout], F32, tag="gemm_ps")
                        for k in range(kch):
                            nc.tensor.matmul(
                                ps[:],
                                xt_sb[k][:, r * 128:r * 128 + rw],
                                w_sb[:xt_sb[k].shape[0], k * dout:(k + 1) * dout],
                                start=(k == 0),
                                stop=(k == kch - 1),
                            )
                        nc.scalar.activation(
                        xw_stage[:rw, r * dout:(r + 1) * dout], ps[:],
                        mybir.ActivationFunctionType.Copy)
                    # store xw_shard (row-major) then AllGather
                    nfull = shard // 128
                    nc.sync.dma_start(
                        t_xw_shard[: nfull * 128, :].rearrange("(r p) e -> p r e", p=128),
                        xw_stage[:, : nfull * dout].rearrange("p (r e) -> p r e", e=dout),
                    )
                    if shard > nfull * 128:
                        rw = shard - nfull * 128
                        nc.sync.dma_start(
                            t_xw_shard[nfull * 128:, :],
                            xw_stage[:rw, nfull * dout:(nfull + 1) * dout],
                        )
                    nc.gpsimd.collective_compute(
                        "AllGather",
                        mybir.AluOpType.bypass,
                        replica_groups=[list(range(cores))],
                        ins=[t_xw_shard[:]],
                        outs=[t_xw_full[:]],
                    )

                # ---- scatter phase ----
                in_ap = {}
                if nlo:
                    in_ap["lo"] = t_xw_full[0:half, :]
                if nhi:
                    in_ap["hi"] = t_xw_full[half:n_nodes, :]
                gtot = {"lo": gtot_lo, "hi": gtot_hi}
                nbatch = {s: _cdiv(gtot[s], maxb) for s in in_ap}
                pool = {"lo": mlp, "hi": mhp}
                msgs_buf = {s: [None] * nbatch[s] for s in in_ap}
                qctr = [0]
                noh = {s: _cdiv(gtot[s], max(ohk, 1)) for s in in_ap}
                oh_buf = {s: [None] * noh[s] for s in in_ap}
                scaled = {s: [False] * nbatch[s] for s in in_ap}

                def emit_oh(s, c):
                    g0 = c * ohk
                    gn = min(ohk, gtot[s] - g0)
                    buf = ohp.tile([128, gn * win], F32, tag="oh",
                                   name=f"oh_{s}_{c}_r{rep}")
                    nc.vector.tensor_tensor(
                        buf[:].rearrange("p (k j) -> p k j", j=win),
                        iota_f[:, :gn * win].rearrange("p (k j) -> p k j", j=win),
                        sb_dst[s][:, g0:g0 + gn].broadcast_to([128, gn, win]),
                        op=mybir.AluOpType.is_equal,
                    )
                    oh_buf[s][c] = buf

                def emit_gather(s, b):
                    g0 = b * maxb
                    gn = min(maxb, gtot[s] - g0)
                    n_idx = gn * 128
                    buf = pool[s].tile([128, gn * dout], F32, tag=f"msgs_{s}", name=f"msgs_{s}_{b}_r{rep}")
                    nc.gpsimd.dma_gather(
                        buf[:].rearrange("p (c e) -> p c e", e=dout),
                        in_ap[s],
                        sb_idx[s][:, g0 * 8:(g0 + gn) * 8],
                        n_idx,
                        n_idx,
                        dout,
                        single_packet=False,
                        queue_num=qctr[0] % 4,
                    )
                    qctr[0] += 1
                    msgs_buf[s][b] = buf

                def emit_scale(s, b):
                    # fold edge values into the messages: one bulk multiply.
                    # Deferred to first consumption so the DVE doesn't
                    # head-of-line block on the gather DMA.
                    g0 = b * maxb
                    gn = min(maxb, gtot[s] - g0)
                    buf = msgs_buf[s][b]
                    nc.vector.tensor_tensor(
                        buf[:].rearrange("p (c e) -> p c e", e=dout),
                        buf[:].rearrange("p (c e) -> p c e", e=dout),
                        sb_val[s][:, g0:g0 + gn].broadcast_to([128, gn, dout]),
                        op=mybir.AluOpType.mult,
                    )
                    scaled[s][b] = True

                out_stage = stagep.tile([dout, shard], F32, tag="out_stage")
                cum = {"lo": cum_lo, "hi": cum_hi}
                if debug_skip_scatter == "gather_only":
                    # gathers only; consume each batch with one cheap DVE add
                    acc = stagep.tile([128, dout], F32, tag="dbg_acc")
                    nc.vector.memset(acc[:], 0.0)
                    for s in in_ap:
                        for b in range(nbatch[s]):
                            emit_gather(s, b)
                            nc.vector.tensor_tensor(
                                acc[:], acc[:], msgs_buf[s][b][:, :dout],
                                op=mybir.AluOpType.add)
                    nc.vector.tensor_copy(out_stage[:, :dout], acc[:dout, :dout])
                    nc.sync.dma_start(t_out[:], out_stage[:])
                    return
                if debug_skip_scatter:
                    # debug: outT = xw_full[core-shard].T via strided DMA read
                    nc.sync.dma_start(
                        out_stage[:],
                        t_xw_full[0:shard, :].rearrange("n e -> e n"))
                    nc.sync.dma_start(t_out[:], out_stage[:])
                    nwin_eff = 0
                else:
                    nwin_eff = nwin
                for wi in range(nwin_eff):
                    ww = min(win, shard - wi * win)
                    spans = [(s, int(cum[s][wi]), int(cum[s][wi + 1])) for s in in_ap]
                    ngrp = sum(g1 - g0 for _, g0, g1 in spans)
                    if ngrp == 0:
                        # no edges anywhere for this window: bias + relu of zero
                        zps = spsp.tile([dout, win], F32, tag="sc_ps")
                        nc.vector.memset(zps[:], 0.0)
                        nc.scalar.activation(
                            out_stage[:, wi * win:wi * win + ww], zps[:, :ww],
                            mybir.ActivationFunctionType.Relu, bias=bias_sb[:],
                        )
                        continue
                    ps = spsp.tile([dout, win], F32, tag="sc_ps")
                    gi = 0
                    for s, g0, g1 in spans:
                        for g in range(g0, g1):
                            b, j = g // maxb, g % maxb
                            if msgs_buf[s][b] is None:
                                emit_gather(s, b)
                            if debug_mode == "const_oh":
                                oh_ap = iota_f[:, :win]
                            elif ohk == 0:
                                oh = ohp.tile([128, win], F32, tag="oh")
                                if act_build and gi % act_build == act_build - 1:
                                    # scalar-engine build (exact for int iota):
                                    # oh = val * relu(1 - |iota - dst|)
                                    ab = ohp.tile([128, win], F32, tag="abst")
                                    nc.scalar.activation(
                                        ab[:], iota_f[:, :win],
                                        mybir.ActivationFunctionType.Abs,
                                        bias=sb_negd[s][:, g:g + 1])
                                    nc.scalar.activation(
                                        oh[:], ab[:],
                                        mybir.ActivationFunctionType.Relu,
                                        bias=sb_val[s][:, g:g + 1],
                                        scale=sb_negv[s][:, g:g + 1])
                                else:
                                    # per-group fused build on DVE
                                    nc.vector.tensor_scalar(
                                        oh[:],
                                        iota_f[:, :win],
                                        sb_dst[s][:, g:g + 1],
                                        sb_val[s][:, g:g + 1],
                                        mybir.AluOpType.is_equal,
                                        mybir.AluOpType.mult,
                                    )
                                oh_ap = oh[:]
                            else:
                                if not scaled[s][b]:
                                    emit_scale(s, b)
                                oc, ojj = g // ohk, g % ohk
                                if oh_buf[s][oc] is None:
                                    emit_oh(s, oc)
                                oh_ap = oh_buf[s][oc][:, ojj * win:(ojj + 1) * win]
                            if debug_mode == "no_mm":
                                if gi == 0:
                                    nc.tensor.matmul(
                                        ps[:], msgs_buf[s][b][:, j * dout:(j + 1) * dout],
                                        oh_ap, start=True, stop=True)
                            else:
                                nc.tensor.matmul(
                                    ps[:],
                                    msgs_buf[s][b][:, j * dout:(j + 1) * dout],
                                    oh_ap,
                                    start=(gi == 0),
                                    stop=(gi == ngrp - 1),
                                )
                            gi += 1
                    nc.scalar.activation(
                        out_stage[:, wi * win:wi * win + ww], ps[:, :ww],
                        mybir.ActivationFunctionType.Relu, bias=bias_sb[:],
                    )
                nc.sync.dma_start(t_out[:], out_stage[:])

            for _rep in range(repeat):
                emit_body(_rep)

    nc.compile()
    return nc


def run(inputs, *, n_nodes, n_edges, din, dout, cores, win=128, maxb=64,
        msgs_bufs=2, gp_build=0, sc_ps_bufs=4, debug_mode=None,
        ohk=16, oh_bufs=3, act_build=0,
        trace=False, debug_skip_gemm=False, debug_skip_scatter=False,
        repeat=1):
    x = np.ascontiguousarray(np.asarray(inputs["x"], dtype=np.float32))
    weight = np.ascontiguousarray(np.asarray(inputs["weight"], dtype=np.float32))
    bias = np.ascontiguousarray(
        np.asarray(inputs["bias"], dtype=np.float32).reshape(dout, 1))
    meta = preprocess(
        inputs["edge_src"], inputs["edge_dst"], inputs["edge_vals"],
        n_nodes=n_nodes, cores=cores, win=win)
    shard = meta["shard"]

    nc = build_program(meta, n_nodes=n_nodes, din=din, dout=dout, cores=cores,
                       win=win, maxb=maxb, msgs_bufs=msgs_bufs,
                       gp_build=gp_build, sc_ps_bufs=sc_ps_bufs,
                       debug_mode=debug_mode, ohk=ohk, oh_bufs=oh_bufs,
                       act_build=act_build,
                       debug_skip_gemm=debug_skip_gemm,
                       debug_skip_scatter=debug_skip_scatter, repeat=repeat)

    xT = np.ascontiguousarray(x.T)
    in_maps = []
    for c in range(cores):
        m = {
            "xT": np.ascontiguousarray(xT[:, c * shard:(c + 1) * shard]),
            "w": weight,
            "bias": bias,
        }
        if meta["nlo"]:
            m["idx_lo"] = meta["idx_lo"][c]
            m["dst_lo"] = meta["dst_lo"][c]
            m["val_lo"] = meta["val_lo"][c]
        if meta["nhi"]:
            m["idx_hi"] = meta["idx_hi"][c]
            m["dst_hi"] = meta["dst_hi"][c]
            m["val_hi"] = meta["val_hi"][c]
        if debug_skip_gemm:
            m["xw_full"] = np.ascontiguousarray(x @ weight)
        in_maps.append(m)

    res = bass_utils.run_bass_kernel_spmd(
        nc, in_maps, core_ids=list(range(cores)), trace=trace)
    out = np.concatenate(
        [res.results[c]["outT"].T for c in range(cores)], axis=0)
    run.last_nc = nc
    run.last_in_maps = in_maps
    return out, res


def kernel(**inputs):
    out, _ = run(
        inputs,
        n_nodes=50000, n_edges=800000, din=256, dout=64, cores=8,
        maxb=48, msgs_bufs=4, ohk=0, oh_bufs=8,
    )
    return np.ascontiguousarray(out, dtype=np.float32)

